# revision 3
# baseline (speedup 1.0000x reference)
"""Trainium2 Bass kernel for the max-plus (tropical) forward-backward chain.

Problem: 2-state max-plus message passing over length-4096 chains, batch 2048.
    psi = j * outer([-.5,.5],[-.5,.5]);  phi[b,i,s] = b[obs[b,i]] * values[s]
    forward/backward max-plus scans + belief assembly.

Algorithm (exact reduction, valid for j > 0 and each b_s either >= 0 or
<= -j; holds for the staged inputs):
  The message difference d = msg[1]-msg[0] follows a clamped walk
  d' = clamp(d + tau_i, -j/2, +j/2), tau_i = b[obs_i].  With e = -d this is
  e' = max(A(obs), e) + B(obs) -- exactly one hardware
  tensor_tensor_scan(max, add) per direction.  The message level
  msg[0] accumulates v_i = (a - c_i) + rho_i with rho_i =
  relu(-e_i - Omega(obs_i)), Omega = j/2 - tau.  Combining forward level
  Rf (prefix sum) and backward level Rb (suffix sum) into
  H_i = Rf_i + Rb_i gives the single recurrence
  H_i = (H_{i-1} + rho_f_{i-1}) - rho_b_i  (one more scan), and
    out0 = H + K1,  out1 = H + K2 + v1,  v1 = -Omega - (ef + eb)
  with per-row scalars K1 = 0.5*sum(Omega) - j/4, K2 = K1 + j/2.

Sharding: pure data parallel - batch 2048 -> 8 cores x 256 sequences
(2 partition-tiles of 128 each); the L-scans stay on-device per core.
"""

import sys

import numpy as np

if "/opt/trn_rl_repo" not in sys.path:
    sys.path.insert(0, "/opt/trn_rl_repo")

B_FULL, L_FULL, N_CORES = 2048, 4096, 8

_PROGRAM_CACHE = {}


def _build_program(j, b0, b1, B_c, L):
    """Build + compile the per-core program: obs int8 [B_c, L] -> out f32 [B_c,2,L]."""
    import concourse.bacc as bacc
    import concourse.mybir as mybir
    import concourse.tile as tile

    f32 = mybir.dt.float32
    Alu = mybir.AluOpType
    Act = mybir.ActivationFunctionType

    a = j / 4.0
    twoa = j / 2.0
    taus = [b0, b1]
    As, Bs = [], []
    for t in taus:
        if t >= 0:
            As.append(t - twoa)
            Bs.append(-t)
        else:  # t <= -j: guaranteed reset to the bottom clamp
            As.append(twoa)
            Bs.append(0.0)
    sA, bA = As[1] - As[0], As[0]
    sB, bB = Bs[1] - Bs[0], Bs[0]
    # Omega = twoa - tau(o) = (twoa - b0) - (b1 - b0) * o
    sOm, bOm = -(b1 - b0), twoa - b0

    n_tiles = B_c // 128
    assert B_c % 128 == 0

    nc = bacc.Bacc("TRN2", target_bir_lowering=False, debug=False)
    obs_d = nc.dram_tensor("obs", [B_c, L], mybir.dt.int8, kind="ExternalInput").ap()
    out_d = nc.dram_tensor("out", [B_c, 2, L], f32, kind="ExternalOutput").ap()

    with tile.TileContext(nc) as tc:
        with tc.tile_pool(name="const", bufs=1) as cpool, tc.tile_pool(
            name="work", bufs=1
        ) as pool:
            bA_t = cpool.tile([128, 1], f32, tag="bA")
            bB_t = cpool.tile([128, 1], f32, tag="bB")
            bOm_t = cpool.tile([128, 1], f32, tag="bOm")
            nc.vector.memset(bA_t[:], bA)
            nc.vector.memset(bB_t[:], bB)
            nc.vector.memset(bOm_t[:], bOm)

            for t in range(n_tiles):
                rows = slice(t * 128, (t + 1) * 128)
                o = pool.tile([128, L], mybir.dt.int8, tag="o")
                nc.sync.dma_start(out=o[:], in_=obs_d[rows, :])

                A = pool.tile([128, L], f32, tag="A")
                B = pool.tile([128, L], f32, tag="B")
                Om = pool.tile([128, L], f32, tag="Om")
                sOm_t = pool.tile([128, 1], f32, tag="sOm")
                nc.scalar.activation(A[:], o[:], Act.Identity, bias=bA_t[:], scale=sA)
                nc.scalar.activation(B[:], o[:], Act.Identity, bias=bB_t[:], scale=sB)
                nc.scalar.activation(
                    Om[:], o[:], Act.Identity, bias=bOm_t[:], scale=sOm,
                    accum_out=sOm_t[:],
                )

                efb = pool.tile([128, L + 1], f32, tag="efb")
                ebb = pool.tile([128, L + 1], f32, tag="ebb")
                nc.vector.memset(efb[:, 0:1], 0.0)
                nc.vector.memset(ebb[:, 0:1], 0.0)
                # forward walk: state = max(A_t, state) + B_t
                nc.vector.tensor_tensor_scan(
                    efb[:, 1 : L + 1], A[:], B[:], 0.0, Alu.max, Alu.add
                )
                # backward walk on reversed inputs
                nc.vector.tensor_tensor_scan(
                    ebb[:, 1 : L + 1], A[:, ::-1], B[:, ::-1], 0.0, Alu.max, Alu.add
                )

                # rho_f = relu(-ef_sh - Om) into rfb[:,1:]; rfb[:,0] = 0
                xf = pool.tile([128, L], f32, tag="xf")
                nc.vector.scalar_tensor_tensor(
                    xf[:], efb[:, 0:L], -1.0, Om[:], Alu.mult, Alu.subtract
                )
                rfb = pool.tile([128, L + 1], f32, tag="rfb")
                nc.vector.memset(rfb[:, 0:1], 0.0)
                nc.scalar.activation(rfb[:, 1 : L + 1], xf[:], Act.Relu)

                # rho_b (reversed layout) = relu(-eb_sh - Om_rev), accum -> totrb
                xb = pool.tile([128, L], f32, tag="xb")
                nc.vector.scalar_tensor_tensor(
                    xb[:], ebb[:, 0:L], -1.0, Om[:, ::-1], Alu.mult, Alu.subtract
                )
                rbb = pool.tile([128, L], f32, tag="rbb")
                totrb = pool.tile([128, 1], f32, tag="totrb")
                nc.scalar.activation(rbb[:], xb[:], Act.Relu, accum_out=totrb[:])

                # H scan: H_i = (rho_f_{i-1} + H_{i-1}) - rho_b_i, init = sum(rho_b)
                Hb = pool.tile([128, L], f32, tag="Hb")
                nc.vector.tensor_tensor_scan(
                    Hb[:], rfb[:, 0:L], rbb[:, ::-1], totrb[:, 0:1],
                    Alu.add, Alu.subtract,
                )

                # u = ef_sh + eb_fwd ; v1 = -Om - u
                u = pool.tile([128, L], f32, tag="xf")
                nc.vector.tensor_tensor(
                    u[:], efb[:, 0:L], ebb[:, 0:L][:, ::-1], Alu.add
                )
                v1 = pool.tile([128, L], f32, tag="xb")
                nc.vector.scalar_tensor_tensor(
                    v1[:], Om[:], -1.0, u[:], Alu.mult, Alu.subtract
                )

                # K1 = 0.5*sOm - a ; K2 = 0.5*sOm + a
                K1 = pool.tile([128, 1], f32, tag="K1")
                K2 = pool.tile([128, 1], f32, tag="K2")
                nc.vector.tensor_scalar(K1[:], sOm_t[:], 0.5, -a, Alu.mult, Alu.add)
                nc.vector.tensor_scalar(K2[:], sOm_t[:], 0.5, a, Alu.mult, Alu.add)

                out0 = pool.tile([128, L], f32, tag="A")
                nc.vector.tensor_scalar(
                    out0[:], Hb[:], K1[:, 0:1], None, Alu.add
                )
                out1 = pool.tile([128, L], f32, tag="B")
                nc.vector.scalar_tensor_tensor(
                    out1[:], Hb[:], K2[:, 0:1], v1[:], Alu.add, Alu.add
                )

                nc.sync.dma_start(out=out_d[rows, 0, :], in_=out0[:])
                nc.sync.dma_start(out=out_d[rows, 1, :], in_=out1[:])

    nc.compile()
    return nc


def _get_program(j, b0, b1, B_c, L):
    key = (float(j), float(b0), float(b1), B_c, L)
    if key not in _PROGRAM_CACHE:
        _PROGRAM_CACHE[key] = _build_program(j, b0, b1, B_c, L)
    return _PROGRAM_CACHE[key]


def _reference_np(j, b, observations):
    """Literal numpy fallback for parameter regimes the fast path can't handle."""
    j = np.float32(np.asarray(j).reshape(-1)[0])
    b = np.asarray(b, np.float32)
    obs = np.asarray(observations)
    B, L = obs.shape
    values = np.array([-0.5, 0.5], np.float32)
    psi = j * values[:, None] * values[None, :]
    phi = b[obs][..., None] * values

    def step(msg, phi_i):
        tmp = phi_i[:, :, None] + psi[None, :, :] + msg[:, :, None]
        return tmp.max(axis=1).astype(np.float32)

    fwd = np.zeros((B, L, 2), np.float32)
    msg = np.zeros((B, 2), np.float32)
    for i in range(L - 1):
        msg = step(msg, phi[:, i])
        fwd[:, i + 1] = msg
    bwd = np.zeros((B, L, 2), np.float32)
    msg = np.zeros((B, 2), np.float32)
    for i in range(L - 1, 0, -1):
        msg = step(msg, phi[:, i])
        bwd[:, i - 1] = msg
    return np.ascontiguousarray(
        (phi + fwd + bwd).transpose(0, 2, 1).astype(np.float32)
    )


TRACE = False
LAST_RESULTS = None


def kernel(j, b, observations):
    from concourse.bass_utils import run_bass_kernel_spmd

    j_np = np.asarray(j, np.float32).reshape(-1)
    b_np = np.asarray(b, np.float32).reshape(-1)
    obs = np.asarray(observations)
    jf, b0, b1 = float(j_np[0]), float(b_np[0]), float(b_np[1])

    fast = jf > 0 and all(t >= 0 or t <= -jf for t in (b0, b1))
    if not fast:
        return _reference_np(j, b, observations)

    B, L = obs.shape
    B_c = B // N_CORES
    nc = _get_program(jf, b0, b1, B_c, L)

    obs8 = np.ascontiguousarray(obs.astype(np.int8))
    in_maps = [
        {"obs": obs8[c * B_c : (c + 1) * B_c]} for c in range(N_CORES)
    ]
    res = run_bass_kernel_spmd(
        nc, in_maps, core_ids=list(range(N_CORES)), trace=TRACE
    )
    global LAST_RESULTS
    LAST_RESULTS = res
    return np.concatenate([r["out"] for r in res.results], axis=0)


# revision 4
# speedup vs baseline: 1.2236x; 1.2236x over previous
"""Trainium2 Bass kernel for the max-plus (tropical) forward-backward chain.

Problem: 2-state max-plus message passing over length-4096 chains, batch 2048.
    psi = j * outer([-.5,.5],[-.5,.5]);  phi[b,i,s] = b[obs[b,i]] * values[s]
    forward/backward max-plus message scans + belief assembly.

Algorithm (exact reduction; valid for j > 0 and each b_s either >= 0 or
<= -j, which holds for the staged inputs):
  The message difference d = msg[1]-msg[0] follows a clamped walk
  d' = clamp(d + tau_i, -j/2, +j/2), tau_i = b[obs_i].  With e = -d this is
  e' = max(A(obs), e) + B(obs): one hardware tensor_tensor_scan(max, add)
  per direction.  The message level msg[0] accumulates
  v_i = (a - c_i) + rho_i,  rho_i = relu(-e_i - (j/2 - tau_i)).
  Forward prefix Rf + backward suffix Rb combine into one more scan:
  H_i = (rho_f_{i-1} + H_{i-1}) - rho_b_i, seeded with sum(rho_b) + K1 so the
  scan emits out0 = H + K1 directly.  Then
    out1 = (out0 + b0) + v1,   v1 = beta*o - ef_sh - eb_rev  (beta = b1-b0)
  using per-row scalar K1 = 0.5*sum(j/2 - tau) - j/4 derived from the
  accumulated sum of the A tile.

Sharding: pure data parallel - batch 2048 -> 8 cores x 256 sequences
(2 partition-tiles of 128); the L-scans stay on-device per core.

Engine split per [128, 4096] tile:
  DVE    : 3 scans (ef, eb, H/out0) + 2 stt (xf, xb) + v1 TT + out1 stt
  ScalarE: A build (int8->f32 affine, accum for K1), 2 relus
  GpSimd : B build (int8->f32 affine), pad-column memsets
  sync   : DMAs
"""

import sys

import numpy as np

if "/opt/trn_rl_repo" not in sys.path:
    sys.path.insert(0, "/opt/trn_rl_repo")

B_FULL, L_FULL, N_CORES = 2048, 4096, 8

_PROGRAM_CACHE = {}


def _build_program(j, b0, b1, B_c, L):
    """Build + compile the per-core program: obs int8 [B_c, L] -> out f32 [B_c,2,L]."""
    import concourse.bacc as bacc
    import concourse.mybir as mybir
    import concourse.tile as tile

    f32 = mybir.dt.float32
    Alu = mybir.AluOpType
    Act = mybir.ActivationFunctionType

    a = j / 4.0
    twoa = j / 2.0
    beta = b1 - b0
    taus = [b0, b1]
    As, Bs = [], []
    for t in taus:
        if t >= 0:
            As.append(t - twoa)  # only the upper clamp can bind
            Bs.append(-t)
        else:  # t <= -j: guaranteed reset to the bottom clamp
            As.append(twoa)
            Bs.append(0.0)
    sA, bA = As[1] - As[0], As[0]
    sB, bB = Bs[1] - Bs[0], Bs[0]
    assert abs(sA) > 1e-8  # caller guarantees (K1 derives from sum(A))
    # K1 = 0.5*sum(Omega) - a, Omega = twoa - tau:
    #   sumOm = twoa*L - (b0*L + beta*N1), N1 = (sumA - bA*L)/sA
    # => K1 = cK0 + cK1 * sumA
    cK1 = -0.5 * beta / sA
    cK0 = 0.5 * (twoa * L - b0 * L + beta * bA * L / sA) - a
    bR = -(twoa - b0)  # relu bias: rho = relu(xf + bR), xf = beta*o - e_sh

    n_tiles = B_c // 128
    assert B_c % 128 == 0

    nc = bacc.Bacc("TRN2", target_bir_lowering=False, debug=False)
    obs_d = nc.dram_tensor("obs", [B_c, L], mybir.dt.int8, kind="ExternalInput").ap()
    out_d = nc.dram_tensor("out", [B_c, 2, L], f32, kind="ExternalOutput").ap()

    with tile.TileContext(nc) as tc:
        with tc.tile_pool(name="const", bufs=1) as cpool, tc.tile_pool(
            name="work", bufs=1
        ) as pool, tc.tile_pool(name="inp", bufs=2) as ipool:
            bA_t = cpool.tile([128, 1], f32, tag="bA")
            bR_t = cpool.tile([128, 1], f32, tag="bR")
            nc.vector.memset(bA_t[:], bA)
            nc.vector.memset(bR_t[:], bR)

            for t in range(n_tiles):
                rows = slice(t * 128, (t + 1) * 128)
                o = ipool.tile([128, L], mybir.dt.int8, tag="o")
                nc.sync.dma_start(out=o[:], in_=obs_d[rows, :])

                A = pool.tile([128, L], f32, tag="A")
                B = pool.tile([128, L], f32, tag="B")
                sA_t = pool.tile([128, 1], f32, tag="sA")
                nc.scalar.activation(
                    A[:], o[:], Act.Identity, bias=bA_t[:], scale=sA,
                    accum_out=sA_t[:],
                )
                nc.gpsimd.tensor_scalar(B[:], o[:], sB, bB, Alu.mult, Alu.add)

                efb = pool.tile([128, L + 1], f32, tag="efb")
                ebb = pool.tile([128, L + 1], f32, tag="ebb")
                nc.gpsimd.memset(efb[:, 0:1], 0.0)
                nc.gpsimd.memset(ebb[:, 0:1], 0.0)
                # forward walk: state = max(A_t, state) + B_t
                nc.vector.tensor_tensor_scan(
                    efb[:, 1 : L + 1], A[:], B[:], 0.0, Alu.max, Alu.add
                )
                # backward walk on reversed inputs
                nc.vector.tensor_tensor_scan(
                    ebb[:, 1 : L + 1], A[:, ::-1], B[:, ::-1], 0.0, Alu.max, Alu.add
                )

                # xf = beta*o - ef_sh ; rho_f = relu(xf + bR) into rfb[:,1:]
                xf = pool.tile([128, L], f32, tag="xf")
                nc.vector.scalar_tensor_tensor(
                    xf[:], o[:], beta, efb[:, 0:L], Alu.mult, Alu.subtract
                )
                rfb = pool.tile([128, L + 1], f32, tag="rfb")
                nc.gpsimd.memset(rfb[:, 0:1], 0.0)
                nc.scalar.activation(rfb[:, 1 : L + 1], xf[:], Act.Relu, bias=bR_t[:])

                # xb (reversed layout) = beta*o_rev - eb_sh; rho_b = relu(+bR)
                xb = pool.tile([128, L], f32, tag="xb")
                nc.vector.scalar_tensor_tensor(
                    xb[:], o[:, ::-1], beta, ebb[:, 0:L], Alu.mult, Alu.subtract
                )
                rbb = pool.tile([128, L], f32, tag="rbb")
                totrb = pool.tile([128, 1], f32, tag="totrb")
                nc.scalar.activation(
                    rbb[:], xb[:], Act.Relu, bias=bR_t[:], accum_out=totrb[:]
                )

                # scan seed = totrb + K1,  K1 = cK0 + cK1*sumA   (tiny [128,1] ops)
                K1 = pool.tile([128, 1], f32, tag="K1")
                nc.vector.tensor_scalar(K1[:], sA_t[:], cK1, cK0, Alu.mult, Alu.add)
                seed = pool.tile([128, 1], f32, tag="seed")
                nc.vector.tensor_tensor(seed[:], K1[:], totrb[:], Alu.add)

                # H scan emits out0 directly: state=(rho_f_sh + state) - rho_b_fwd
                out0 = pool.tile([128, L], f32, tag="out0")
                nc.vector.tensor_tensor_scan(
                    out0[:], rfb[:, 0:L], rbb[:, ::-1], seed[:, 0:1],
                    Alu.add, Alu.subtract,
                )

                # v1 = xf - eb_rev ; out1 = (out0 + b0) + v1
                v1 = pool.tile([128, L], f32, tag="v1")
                nc.vector.tensor_tensor(
                    v1[:], xf[:], ebb[:, 0:L][:, ::-1], Alu.subtract
                )
                out1 = pool.tile([128, L], f32, tag="out1")
                nc.vector.scalar_tensor_tensor(
                    out1[:], out0[:], float(b0), v1[:], Alu.add, Alu.add
                )

                nc.sync.dma_start(out=out_d[rows, 0, :], in_=out0[:])
                nc.sync.dma_start(out=out_d[rows, 1, :], in_=out1[:])

    nc.compile()
    return nc


def _get_program(j, b0, b1, B_c, L):
    key = (float(j), float(b0), float(b1), B_c, L)
    if key not in _PROGRAM_CACHE:
        _PROGRAM_CACHE[key] = _build_program(j, b0, b1, B_c, L)
    return _PROGRAM_CACHE[key]


def _reference_np(j, b, observations):
    """Literal numpy fallback for parameter regimes the fast path can't handle."""
    j = np.float32(np.asarray(j).reshape(-1)[0])
    b = np.asarray(b, np.float32)
    obs = np.asarray(observations)
    B, L = obs.shape
    values = np.array([-0.5, 0.5], np.float32)
    psi = j * values[:, None] * values[None, :]
    phi = b[obs][..., None] * values

    def step(msg, phi_i):
        tmp = phi_i[:, :, None] + psi[None, :, :] + msg[:, :, None]
        return tmp.max(axis=1).astype(np.float32)

    fwd = np.zeros((B, L, 2), np.float32)
    msg = np.zeros((B, 2), np.float32)
    for i in range(L - 1):
        msg = step(msg, phi[:, i])
        fwd[:, i + 1] = msg
    bwd = np.zeros((B, L, 2), np.float32)
    msg = np.zeros((B, 2), np.float32)
    for i in range(L - 1, 0, -1):
        msg = step(msg, phi[:, i])
        bwd[:, i - 1] = msg
    return np.ascontiguousarray(
        (phi + fwd + bwd).transpose(0, 2, 1).astype(np.float32)
    )


TRACE = False
LAST_RESULTS = None


def kernel(j, b, observations):
    from concourse.bass_utils import run_bass_kernel_spmd

    j_np = np.asarray(j, np.float32).reshape(-1)
    b_np = np.asarray(b, np.float32).reshape(-1)
    obs = np.asarray(observations)
    jf, b0, b1 = float(j_np[0]), float(b_np[0]), float(b_np[1])

    fast = (
        jf > 0
        and all(t >= 0 or t <= -jf for t in (b0, b1))
        and abs(b0 - jf) > 1e-8 * max(1.0, jf)  # K1 derivation needs sA != 0
    )
    if not fast:
        return _reference_np(j, b, observations)

    B, L = obs.shape
    B_c = B // N_CORES
    nc = _get_program(jf, b0, b1, B_c, L)

    obs8 = np.ascontiguousarray(obs.astype(np.int8))
    in_maps = [
        {"obs": obs8[c * B_c : (c + 1) * B_c]} for c in range(N_CORES)
    ]
    res = run_bass_kernel_spmd(
        nc, in_maps, core_ids=list(range(N_CORES)), trace=TRACE
    )
    global LAST_RESULTS
    LAST_RESULTS = res
    return np.concatenate([r["out"] for r in res.results], axis=0)
